# revision 45
# baseline (speedup 1.0000x reference)
"""Trainium2 Bass kernel: masked dot-product attention, one head per NeuronCore.

Reference computation (per head):
    S = Q @ K^T * (1/sqrt(d))           [q, s]
    S = where(mask, S, -1e9)
    P = softmax(S, axis=-1)
    O = P @ V                            [q, v]

Device algorithm (per core, head-parallel across 8 cores, everything kept in
"transposed" [s, q] layout so the P@V matmul needs no on-chip transpose):
    S~^T  = K @ Q'^T + 1.0*mask^T        TensorE; Q' prescaled by scale/ALPHA on
                                         host, mask^T streamed as fp8 {0,1} and
                                         added via an identity-weight matmul
                                         into the same PSUM accumulation.
    E^T   = exp(ALPHA*S~^T - ALPHA)      ScalarE (one pass over all scores).
                                         kept:   exp(scale*S)
                                         masked: exp(scale*S - ALPHA) ~= 0
    [O^T; den] = [V | 1]^T @ E^T         TensorE, bf16, PSUM accumulation over s.
    O^T  *= 1/den                        VectorE reciprocal + GpSimd partition
                                         broadcast + VectorE multiply.
Host does layout-only work: transposes, dtype casts, mask tiling, final
O^T -> O transpose.  No exp-max subtraction is needed: scores are ~N(0,1)
after scaling, so exp stays in [e^-70, e^6] which fp32/bf16 handle exactly.
"""

import math
import sys

import numpy as np

_TRN_REPO = "/opt/trn_rl_repo"
if _TRN_REPO not in sys.path:
    sys.path.insert(0, _TRN_REPO)

import ml_dtypes  # noqa: E402

import concourse.bass as bass  # noqa: E402
import concourse.bacc as bacc  # noqa: E402
import concourse.tile as tile  # noqa: E402
from concourse import mybir  # noqa: E402

# Problem geometry (hardcoded per contest rules; builder is parametric for tests)
N_HEADS = 8
SEQ_Q = 4096
SEQ_S = 4096
D_HEAD = 64
V_HEAD = 64

SCALE = 1.0 / math.sqrt(D_HEAD)
ALPHA = 60.0  # masked-score suppression: exp(x - ALPHA) ~ 0

SCH = 128  # s-chunk (psum partition dim / matmul contraction for PV)
QBLK = 512  # matmul moving free dim / one fp32 psum bank
QGRP = 1024  # q columns per outer block (2 psum banks per ScalarE op)
OCT = 8  # s-chunks per mask DMA (1 MiB transfers)

_F32 = mybir.dt.float32
_BF16 = mybir.dt.bfloat16
_FP8 = mybir.dt.float8e4

_NP_BF16 = mybir.dt.np(_BF16)
_NP_FP8 = mybir.dt.np(_FP8)


def build_nc(q_len=SEQ_Q, s_len=SEQ_S, d=D_HEAD, v=V_HEAD):
    """Build the single-core Bass graph (SPMD: same graph on every core)."""
    assert q_len % QGRP == 0 and s_len % (2 * SCH) == 0
    assert d == 64 and v == 64
    n_sc = s_len // SCH
    n_grp = q_len // QGRP
    nb = QGRP // QBLK  # q-blocks per group (2)
    n_pair = n_sc // 2
    oct_sz = min(OCT, n_sc)
    n_oct = n_sc // oct_sz
    nomask = nomm1 = nomm2 = False
    exp_fn = mybir.ActivationFunctionType.Exp

    # all static operands live in one packed f32 tensor -> one DMA, one sem
    vpw = n_sc * (v + 1) // 2  # vp as f32 columns (bf16 pairs)
    o_kt = q_len // 2  # qt/kt stored bf16: half-width in f32 columns
    o_vp = o_kt + s_len // 2
    o_id = o_vp + vpw
    o_bias = o_id + SCH // 4
    o_zero = o_bias + 1
    o_idf = o_zero + 1
    n_static = o_idf + SCH
    nc = bacc.Bacc("TRN2")
    statics = nc.dram_tensor("statics", (SCH, n_static), _F32, kind="ExternalInput")
    mt = nc.dram_tensor("mt", (n_grp, n_sc, SCH, QGRP), _FP8, kind="ExternalInput")
    out_d = nc.dram_tensor("out", (q_len, v), _F32, kind="ExternalOutput")

    with tile.TileContext(nc) as tc:
        with (
            tc.tile_pool(name="singles", bufs=1) as singles,
            tc.tile_pool(name="mpool", bufs=4) as mpool,
            tc.tile_pool(name="epool", bufs=6) as epool,
            tc.tile_pool(name="opool", bufs=2) as opool,
            tc.tile_pool(name="small", bufs=4) as small,
            tc.tile_pool(name="psS", bufs=3, space=bass.MemorySpace.PSUM) as psS,
            tc.tile_pool(name="psA", bufs=2, space=bass.MemorySpace.PSUM) as psA,
        ):
            # statics split into first-use-ordered tiles so PE starts after
            # ~1.5 MB instead of the full 4.8 MB
            KT_HEAD = min(8, n_sc)  # kt chunks in the head slice
            kt_sb0 = singles.tile([SCH, KT_HEAD * 64], _F32)
            nc.sync.dma_start(
                out=kt_sb0, in_=statics[:, o_kt : o_kt + KT_HEAD * 64]
            )
            qt_sb0 = singles.tile([SCH, min(QGRP // 2, q_len // 2)], _F32)
            nc.sync.dma_start(
                out=qt_sb0, in_=statics[:, 0 : min(QGRP // 2, q_len // 2)]
            )
            misc_sb = singles.tile([SCH, n_static - o_vp], _F32)
            nc.sync.dma_start(out=misc_sb, in_=statics[:, o_vp:])
            kt_sb1 = None
            if n_sc > KT_HEAD:
                kt_sb1 = singles.tile([SCH, (n_sc - KT_HEAD) * 64], _F32)
                nc.scalar.dma_start(
                    out=kt_sb1,
                    in_=statics[:, o_kt + KT_HEAD * 64 : o_kt + s_len // 2],
                )
            qt_sb1 = None
            if q_len > QGRP:
                qt_sb1 = singles.tile([SCH, (q_len - QGRP) // 2], _F32)
                nc.scalar.dma_start(
                    out=qt_sb1, in_=statics[:, QGRP // 2 : q_len // 2]
                )

            def qt_slice(rows, q0_, q1_):
                if q0_ < QGRP:
                    return qt_sb0.bitcast(_BF16)[rows, q0_:q1_]
                return qt_sb1.bitcast(_BF16)[rows, q0_ - QGRP : q1_ - QGRP]

            def kt_slice(rows, c0_, c1_):
                if c0_ < KT_HEAD * SCH:
                    return kt_sb0.bitcast(_BF16)[rows, c0_:c1_]
                h = KT_HEAD * SCH
                return kt_sb1.bitcast(_BF16)[rows, c0_ - h : c1_ - h]

            # HAM warmup: dependency-free matmuls fill the dead DMA-gated
            # start window so the PE clock gate is at 8/8 when real work begins
            warm_sb = singles.tile([64, 640], _BF16)
            nc.vector.memset(warm_sb, 0.5)
            wps = psS.tile([SCH, QGRP], _F32, tag="S")
            for _wi in range(20):
                nc.tensor.matmul(
                    wps[:, 0:QBLK],
                    lhsT=warm_sb[:, 0:SCH],
                    rhs=warm_sb[:, SCH : SCH + QBLK],
                    start=True,
                    stop=True,
                    tile_position=(0, 0),
                )
            vp_sb = misc_sb[:, 0:vpw].bitcast(_BF16).rearrange(
                "p (c x) -> p c x", x=v + 1
            )
            id_sb = misc_sb[:, o_id - o_vp : o_id - o_vp + SCH // 4].bitcast(_FP8)
            bias_sb = misc_sb[:, o_bias - o_vp : o_bias - o_vp + 1]
            zero_sb = misc_sb[:, o_zero - o_vp : o_zero - o_vp + 1]
            idf_sb = misc_sb[:, o_idf - o_vp : o_idf - o_vp + SCH]

            for g in range(n_grp):
                q0 = g * QGRP
                acc0 = psA.tile([v + 1, QBLK], _F32, tag="acc", bufs=2)
                acc1 = psA.tile([v + 1, QBLK], _F32, tag="acc", bufs=2)
                accs = [acc0, acc1]
                out_sb = opool.tile([SCH, (QGRP // SCH) * v], _F32, tag="osb")

                ip = 0
                prev = None  # software pipeline: MM2 for pair p-1 issues during pair p
                for oc in range(n_oct):
                    mq = mpool.tile([SCH, oct_sz, QGRP], _FP8, tag="mq")
                    nc.sync.dma_start(
                        out=mq,
                        in_=mt[g, oc * oct_sz : (oc + 1) * oct_sz].rearrange(
                            "c p x -> p c x"
                        ),
                    )
                    for pp in range(oct_sz // 2):
                        sa = oc * oct_sz + 2 * pp
                        sb = sa + 1
                        first = ip == 0
                        last = ip == n_pair - 1
                        # mask via VectorE post-exp multiply for 1 of every 3
                        # pairs; PE identity-matmul otherwise (engine balance)
                        dve_mask = (ip % 2 == 1) and not nomask and not nomm1
                        Sa = psS.tile([SCH, QGRP], _F32, tag="S")
                        Sb = psS.tile([SCH, QGRP], _F32, tag="S")
                        # QK^T: row-packed pair (K=64 each, PE rows 0-63 / 64-127)
                        for qb in range(nb) if not nomm1 else []:
                            cs = slice(qb * QBLK, (qb + 1) * QBLK)
                            qs = slice(q0 + qb * QBLK, q0 + (qb + 1) * QBLK)  # abs bf16 cols
                            nc.tensor.matmul(
                                Sa[:, cs],
                                lhsT=kt_slice(slice(0, d), sa * SCH, (sa + 1) * SCH),
                                rhs=qt_slice(slice(0, d), qs.start, qs.stop),
                                start=True,
                                stop=nomask or dve_mask,
                                tile_position=(0, 0),
                            )
                            nc.tensor.matmul(
                                Sb[:, cs],
                                lhsT=kt_slice(slice(d, 2 * d), sb * SCH, (sb + 1) * SCH),
                                rhs=qt_slice(slice(d, 2 * d), qs.start, qs.stop),
                                start=True,
                                stop=nomask or dve_mask,
                                tile_position=(64, 0),
                            )
                        if not dve_mask and not nomask:
                            for qb in range(nb):
                                cs = slice(qb * QBLK, (qb + 1) * QBLK)
                                nc.tensor.matmul(
                                    Sa[:, cs],
                                    lhsT=id_sb,
                                    rhs=mq[:, 2 * pp, cs],
                                    start=nomm1,
                                    stop=True,
                                )
                                nc.tensor.matmul(
                                    Sb[:, cs],
                                    lhsT=id_sb,
                                    rhs=mq[:, 2 * pp + 1, cs],
                                    start=nomm1,
                                    stop=True,
                                )
                        Ea = epool.tile([SCH, QGRP], _BF16, tag="E")
                        Eb = epool.tile([SCH, QGRP], _BF16, tag="E")
                        if dve_mask:
                            # VectorE adds the mask during the PSUM->SBUF move;
                            # ScalarE exp is identical to the PE-mask path
                            Wa = epool.tile([SCH, QGRP], _F32, tag="W", bufs=4)
                            Wb = epool.tile([SCH, QGRP], _F32, tag="W", bufs=4)
                            nc.vector.tensor_add(Wa, Sa, mq[:, 2 * pp, :])
                            nc.vector.tensor_add(Wb, Sb, mq[:, 2 * pp + 1, :])
                            nc.scalar.activation(
                                out=Ea, in_=Wa, func=exp_fn, bias=bias_sb, scale=ALPHA
                            )
                            nc.scalar.activation(
                                out=Eb, in_=Wb, func=exp_fn, bias=bias_sb, scale=ALPHA
                            )
                        else:
                            nc.scalar.activation(
                                out=Ea, in_=Sa, func=exp_fn, bias=bias_sb, scale=ALPHA
                            )
                            nc.scalar.activation(
                                out=Eb, in_=Sb, func=exp_fn, bias=bias_sb, scale=ALPHA
                            )
                        # [O^T; den] += [V|1]^T @ E^T  (for the PREVIOUS pair,
                        # so PE never stalls waiting on this pair's exp)
                        if prev is not None and not nomm2:
                            psa, psb, pEa, pEb, pfirst = prev
                            for qb in range(nb):
                                cs = slice(qb * QBLK, (qb + 1) * QBLK)
                                nc.tensor.matmul(
                                    accs[qb],
                                    lhsT=vp_sb[:, psa, :],
                                    rhs=pEa[:, cs],
                                    start=pfirst,
                                    stop=False,
                                )
                                nc.tensor.matmul(
                                    accs[qb],
                                    lhsT=vp_sb[:, psb, :],
                                    rhs=pEb[:, cs],
                                    start=False,
                                    stop=False,
                                )
                        prev = (sa, sb, Ea, Eb, first)
                        ip += 1
                # flush the last pair's MM2 (closes the accumulation groups)
                if prev is not None and not nomm2:
                    psa, psb, pEa, pEb, pfirst = prev
                    for qb in range(nb):
                        cs = slice(qb * QBLK, (qb + 1) * QBLK)
                        nc.tensor.matmul(
                            accs[qb],
                            lhsT=vp_sb[:, psa, :],
                            rhs=pEa[:, cs],
                            start=pfirst,
                            stop=False,
                        )
                        nc.tensor.matmul(
                            accs[qb],
                            lhsT=vp_sb[:, psb, :],
                            rhs=pEb[:, cs],
                            start=False,
                            stop=True,
                        )

                # normalize: O^T * (1/den), den = acc row v
                if nomm2:
                    nc.vector.memset(out_sb, 0.0)
                for qb in range(nb) if not nomm2 else []:
                    a = accs[qb]
                    # drain acc psum -> sbuf on ScalarE so the psum slot release
                    # rides the same semaphore MM2 already waits on (one wait)
                    a_sb = small.tile([v + 1, QBLK], _F32, tag="asb", bufs=3)
                    nc.scalar.copy(a_sb, a)
                    # transpose [v+1, 128] chunks -> [128, v+1]: q lands on
                    # partitions so 1/den becomes a per-partition scalar
                    for cc in range(QBLK // SCH):
                        tr = psA.tile([SCH, v + 1], _F32, tag="acc", bufs=2)
                        nc.tensor.transpose(
                            tr,
                            in_=a_sb[:, cc * SCH : (cc + 1) * SCH],
                            identity=idf_sb[0 : v + 1, 0 : v + 1],
                        )
                        rec = small.tile([SCH, 1], _F32, tag="rec")
                        nc.vector.reciprocal(rec, tr[:, v : v + 1])
                        nc.vector.tensor_scalar_mul(
                            out_sb[:, (qb * QBLK // SCH + cc) * v : (qb * QBLK // SCH + cc + 1) * v],
                            in0=tr[:, 0:v],
                            scalar1=rec,
                        )
                nc.sync.dma_start(
                    out=out_d[q0 : q0 + QGRP, :].rearrange("(c p) x -> p c x", p=SCH),
                    in_=out_sb.rearrange("p (c x) -> p c x", x=v),
                )
                if g < n_grp - 1:
                    # boundary warmer: keep the PE clock gate open across the
                    # group epilogue; pinned here via the freed acc slot and
                    # off the critical path (next group starts with MM1/psS)
                    wb = psA.tile([v + 1, QBLK], _F32, tag="acc", bufs=2)
                    for _wi in range(4):
                        nc.tensor.matmul(
                            wb[0:SCH // 2, :],
                            lhsT=warm_sb[:, 0:64],
                            rhs=warm_sb[:, SCH : SCH + QBLK],
                            start=True,
                            stop=True,
                            tile_position=(0, 0),
                        )

    nc.compile()
    return nc


def prep_head(Qh, Kh, Vh, Mh):
    """Host-side layout prep for one head -> the core's input map."""
    q_len, d = Qh.shape
    s_len, v = Vh.shape
    n_sc = s_len // SCH
    n_grp = q_len // QGRP

    vpw = n_sc * (v + 1) // 2
    o_kt = q_len // 2
    o_vp = o_kt + s_len // 2
    o_id = o_vp + vpw
    o_bias = o_id + SCH // 4
    o_zero = o_bias + 1
    o_idf = o_zero + 1
    n_static = o_idf + SCH

    statics = np.zeros((SCH, n_static), dtype=np.float32)
    qt = (np.asarray(Qh, np.float32).T * np.float32(SCALE / ALPHA)).astype(_NP_BF16)
    qt2 = np.ascontiguousarray(np.concatenate([qt, qt], axis=0))  # [128, q] bf16
    statics[:, 0 : q_len // 2] = qt2.view(np.float32)
    kt = np.asarray(Kh, np.float32).T.astype(_NP_BF16)  # [d, s] bf16
    kt2 = np.ascontiguousarray(np.concatenate([kt, kt], axis=0))
    statics[:, o_kt : o_kt + s_len // 2] = kt2.view(np.float32)
    vpad = np.concatenate(
        [np.asarray(Vh, np.float32), np.ones((s_len, 1), np.float32)], axis=1
    )
    vp = vpad.astype(_NP_BF16).reshape(n_sc, SCH, v + 1)  # [c, p, x]
    vp_p = np.ascontiguousarray(vp.transpose(1, 0, 2)).reshape(SCH, n_sc * (v + 1))
    statics[:, o_vp : o_vp + vpw] = vp_p.view(np.float32)
    ident = np.eye(SCH, dtype=np.float32).astype(_NP_FP8)
    statics[:, o_id : o_id + SCH // 4] = ident.view(np.float32)
    statics[:, o_bias] = -ALPHA
    statics[:, o_idf : o_idf + SCH] = np.eye(SCH, dtype=np.float32)

    m = np.asarray(Mh, bool).T.astype(_NP_FP8)  # [s, q] in {0.0, 1.0}
    mt = np.ascontiguousarray(
        m.reshape(n_sc, SCH, n_grp, QGRP).transpose(2, 0, 1, 3)
    )
    return {"statics": statics, "mt": mt}


_NC_CACHE = {}


def get_nc(q_len=SEQ_Q, s_len=SEQ_S, d=D_HEAD, v=V_HEAD):
    key = (q_len, s_len, d, v)
    if key not in _NC_CACHE:
        _NC_CACHE[key] = build_nc(*key)
    return _NC_CACHE[key]


def run_on_device(in_maps, nc=None, trace=False):
    """Run the SPMD kernel on len(in_maps) NeuronCores; returns BassKernelResults."""
    from concourse.bass_utils import run_bass_kernel_spmd

    if nc is None:
        nc = get_nc()
    return run_bass_kernel_spmd(
        nc, in_maps, core_ids=list(range(len(in_maps))), trace=trace
    )


def kernel(queries_nqd, keys_nsd, values_nsv, attention_mask_nqs):
    Q = np.asarray(queries_nqd, dtype=np.float32)
    K = np.asarray(keys_nsd, dtype=np.float32)
    V = np.asarray(values_nsv, dtype=np.float32)
    M = np.asarray(attention_mask_nqs, dtype=bool)
    n, q_len, d = Q.shape
    s_len, v = V.shape[1], V.shape[2]

    nc = get_nc(q_len, s_len, d, v)
    in_maps = [prep_head(Q[i], K[i], V[i], M[i]) for i in range(n)]
    res = run_on_device(in_maps, nc=nc)
    out = np.stack(
        [np.asarray(r["out"], dtype=np.float32) for r in res.results], axis=0
    )
    return np.ascontiguousarray(out)
